# revision 24
# baseline (speedup 1.0000x reference)
"""MoE routing kernel for Trainium2 (8 NeuronCores).

Problem: out[b,l,:] = actions[b,l,:]                      if action_type[b,l] == 0
         out[b,l,:] = W[t-1] @ actions[b,l,:] + b[t-1]    if action_type == t >= 1

Strategy (bf16, balanced expert-split): route each token to the single
expert it needs. The host groups the B*L tokens by action_type and splits
the 7 experts' token sets across all 8 cores: every core runs the same
SPMD program with three token segments A/B/C of 1024/512/384 tokens, each
segment served by its own expert weight (slots of 8/4/3 blocks of 128
tokens; a small DFS packs each expert's blocks into slots). This balances
compute at 1920 tokens/core (vs 2048 with one expert per core and an idle
core 0). Identity-type tokens are copied on the host (exact); tokens that
do not fit the device capacity fall back to host BLAS (rare). If slot
packing is infeasible for a pathological distribution, the kernel falls
back to the one-expert-per-core layout.

Device schedule: token tiles processed in two 512-feature phases of up to
8 PSUM groups [128 tok, 512 feat], accumulated over 8 contraction chunks.
The two-phase split halves the weight bytes the PE demands per matmul
(each phase re-reads x from SBUF), keeping DMA demand below early HBM
supply. Tile sizes decrease (1024/512/256/128) so the final output drain
is short. A warmup chain of dummy matmuls runs during the DMA preamble so
the PE p-state ramp (2x slower until ~3us of continuous execution)
completes before real operands land. Host packs x/w into SBUF-ready
[128, free] bf16 layouts (large contiguous DMAs); weights + steady x
dispatch on the sync queue, first-tile x on scalar. PSUM->SBUF bf16 casts
alternate between the vector and scalar engines; outputs leave as paired
[128, 2048] DMAs (partition-major DRAM layout, host unpacks).
"""

import sys

for _p in ("/root/.axon_site/_ro/trn_rl_repo", "/opt/trn_rl_repo"):
    if _p not in sys.path:
        sys.path.append(_p)

import numpy as np
import ml_dtypes
import concourse.bass as bass
import concourse.tile as tile
from concourse import bacc, mybir
from concourse.bass_utils import run_bass_kernel_spmd

D = 1024
P = 128
N_CORES = 8
FB = 512  # psum feature block (phase width)
NIC = D // P  # 8 contraction chunks
NPH = D // FB  # 2 feature phases
F32 = mybir.dt.float32
BF16 = mybir.dt.bfloat16
BF16NP = ml_dtypes.bfloat16

# Balanced layout: per-core segments (token capacity, weight-segment id).
SLOT_BLOCKS = (8, 4, 3)  # A/B/C slot sizes in 128-token blocks
SEGS = [(0, 1024, 0), (1024, 512, 1), (1536, 256, 2), (1792, 128, 2)]
C_BAL = 1920

_program_cache: dict[tuple, bass.Bass] = {}


def _t_tiles(C):
    """Decreasing tile sizes: big tiles amortize weight streaming early,
    small tiles at the end keep the final output drain short."""
    tiles = []
    t0 = 0
    while t0 < C:
        rest = C - t0
        if rest > 1536:
            tt = 1024
        elif rest > 768:
            tt = 512
        elif rest > 384:
            tt = 256
        else:
            tt = min(P, rest)
        tiles.append((t0, tt, 0))
        t0 += tt
    return tiles


def build_program(tile_spec, n_wseg: int, with_bias: bool) -> bass.Bass:
    """out = x @ w[seg].T per-core, x/w host-packed bf16.

    tile_spec: tuple of (t0, tt, wseg) token tiles.
    DRAM inputs per core:
      xP [P, 8*C]      : cols [(NIC*t0 + ic*tt) ...] hold
                         x.T[ic*128:(ic+1)*128, t0:t0+tt]  (contract chunk
                         ic, token tile [t0, t0+tt)) -- SBUF-ready.
      wP [P, ws*8*D]   : cols [ws*8*D + ph*NIC*FB + ic*FB ...] =
                         w[ws].T[ic*128:(ic+1)*128, ph*FB:(ph+1)*FB]
      bB [P, ws*D]     : broadcast bias rows per segment (if with_bias)
    DRAM output: outP [P, C*D/P] bf16, partition-major: token g*128+p,
    feature f lives at outP[p, g*D + f]. Host unpacks.
    """
    tile_spec = tuple(tile_spec)
    C = sum(tt for _, tt, _ in tile_spec)
    key = (tile_spec, n_wseg, with_bias)
    if key in _program_cache:
        return _program_cache[key]
    nc = bacc.Bacc("TRN2", target_bir_lowering=False, debug=False, num_devices=N_CORES)
    xP = nc.dram_tensor("xP", [P, NIC * C], BF16, kind="ExternalInput")
    wP = nc.dram_tensor("wP", [P, n_wseg * NIC * D], BF16, kind="ExternalInput")
    bB = (
        nc.dram_tensor("bB", [P, n_wseg * D], F32, kind="ExternalInput")
        if with_bias
        else None
    )
    outP = nc.dram_tensor("outP", [P, (C // P) * D], BF16, kind="ExternalOutput")

    PHW = NIC * FB  # wP columns per phase (4096)

    with tile.TileContext(nc) as tc:
        with (
            tc.tile_pool(name="wpool", bufs=1) as wpool,
            tc.tile_pool(name="bpool", bufs=1) as bpool,
            tc.tile_pool(name="xpool", bufs=2) as xpool,
            tc.tile_pool(name="opool", bufs=2) as opool,
            tc.tile_pool(name="psum", bufs=1, space="PSUM") as psum_pool,
        ):
            tt0 = tile_spec[0][1]
            w_tiles = {}  # (wseg, phase, ic) -> (tile, col offset)
            x0_tiles = [None] * NIC

            # PE warmup: dummy matmuls (no DMA deps) during the DMA preamble
            # so the p-state ramp completes before real operands land. Two
            # rotating banks, start/stop once per bank: consecutive
            # same-address starts serialize on the PSUM reset (~430ns).
            warm = wpool.tile([P, FB], BF16, name="warm")
            warm2 = wpool.tile([P, P], BF16, name="warm2")
            nc.gpsimd.memset(warm[:], 0.0)
            nc.gpsimd.memset(warm2[:], 0.0)
            ps_warms = [
                psum_pool.tile([P, FB], F32, name=f"ps_warm{j}", tag=f"ps{6 + j}")
                for j in range(2)
            ]
            NWARM = 6
            for i in range(NWARM):
                nc.tensor.matmul(
                    ps_warms[i % 2][:],
                    warm2[:],
                    warm[:],
                    start=(i < 2),
                    stop=(i >= NWARM - 2),
                )

            def _dma_w(ws, ph, ic0_, nic_):
                wt = wpool.tile(
                    [P, nic_ * FB],
                    BF16,
                    name=f"w{ws}_{ph}_{ic0_}",
                    tag=f"w{ws}_{ph}_{ic0_}",
                )
                base = ws * NIC * D + ph * PHW + ic0_ * FB
                nc.sync.dma_start(wt[:], wP[:, base : base + nic_ * FB])
                for j in range(nic_):
                    w_tiles[(ws, ph, ic0_ + j)] = (wt, j * FB)

            def _dma_x0(ic0_, nic_):
                xt = xpool.tile(
                    [P, nic_ * tt0], BF16, name=f"x0_{ic0_}", tag=f"x0_{ic0_}"
                )
                nc.scalar.dma_start(xt[:], xP[:, ic0_ * tt0 : (ic0_ + nic_) * tt0])
                for j in range(nic_):
                    x0_tiles[ic0_ + j] = (xt, j * tt0)

            # Preamble: interleave segment-0 phase-0 weight and first-tile x
            # chunks so the first matmul needs only ~384KB, then stays fed.
            _dma_w(0, 0, 0, 1)
            _dma_x0(0, 1)
            _dma_w(0, 0, 1, 1)
            _dma_x0(1, 3)
            _dma_w(0, 0, 2, 2)
            _dma_x0(4, 4)
            _dma_w(0, 0, 4, 4)
            _dma_w(0, 1, 0, 4)
            _dma_w(0, 1, 4, 4)
            b_tile = None
            if with_bias:
                b_tile = bpool.tile([P, n_wseg * D], F32, name="b_tile")
                nc.sync.dma_start(b_tile[:], bB[:])

            _x_prefetched = {}
            for ti, (t0, tt, ws) in enumerate(tile_spec):
                ntc = tt // P  # token chunks in this tile (<= 8)
                if ti == 0:
                    xv = x0_tiles
                else:
                    xv = _x_prefetched.pop(ti)

                ot = opool.tile([P, ntc * D], BF16, name=f"ot_{ti}", tag=f"o{ti % 2}")
                for ph in range(NPH):
                    if ti == 0 and ph == 1:
                        # later weight segments: dispatched only now so their
                        # bulk transfers stay clear of the critical early
                        # window (first needed ~27us in)
                        for ws2 in range(1, n_wseg):
                            for ph2 in range(NPH):
                                _dma_w(ws2, ph2, 0, NIC)
                    if ph == 1 and ti + 1 < len(tile_spec):
                        # prefetch next tile's x on the sync queue BEFORE
                        # this tile's out dispatches enter that queue
                        nt0, ntt, _ = tile_spec[ti + 1]
                        xt = xpool.tile(
                            [P, NIC * ntt],
                            BF16,
                            name=f"x{ti + 1}",
                            tag=f"x{(ti + 1) % 2}",
                        )
                        base = NIC * nt0
                        nc.sync.dma_start(xt[:], xP[:, base : base + NIC * ntt])
                        _x_prefetched[ti + 1] = [
                            (xt, j * ntt) for j in range(NIC)
                        ]
                    ps = {
                        c: psum_pool.tile(
                            [P, FB],
                            F32,
                            name=f"ps_{ti}_{ph}_{c}",
                            tag=f"ps{(ph * ntc + c) % 8}",
                        )
                        for c in range(ntc)
                    }
                    for ic in range(NIC):
                        last = ic == NIC - 1
                        wt, woff = w_tiles[(ws, ph, ic)]
                        rhs = wt[:, woff : woff + FB]
                        for c in range(ntc):
                            xt, xoff = xv[ic]
                            nc.tensor.matmul(
                                ps[c][:],
                                xt[:, xoff + c * P : xoff + (c + 1) * P],
                                rhs,
                                start=(ic == 0),
                                stop=last,
                            )
                            if last:
                                # evacuate psum as soon as its group closes,
                                # alternating engines
                                dst = ot[:, c * D + ph * FB : c * D + (ph + 1) * FB]
                                if with_bias:
                                    nc.vector.tensor_add(
                                        dst,
                                        ps[c][:],
                                        b_tile[:, ws * D + ph * FB : ws * D + (ph + 1) * FB],
                                    )
                                elif c % 2 == 0:
                                    nc.vector.tensor_copy(dst, ps[c][:])
                                else:
                                    nc.scalar.copy(dst, ps[c][:])
                                if ph == NPH - 1 and c % 2 == 1:
                                    # store chunk pair (c-1, c)
                                    g = t0 // P + c - 1
                                    nc.sync.dma_start(
                                        outP[:, g * D : (g + 2) * D],
                                        ot[:, (c - 1) * D : (c + 1) * D],
                                    )
                    if ph == NPH - 1 and ntc % 2 == 1:
                        g = t0 // P + ntc - 1
                        nc.sync.dma_start(
                            outP[:, g * D : (g + 1) * D],
                            ot[:, (ntc - 1) * D : ntc * D],
                        )
    nc.compile()
    _program_cache[key] = nc
    return nc


def _pack_x(flat_rows: np.ndarray, tile_spec) -> np.ndarray:
    """[n, D] fp32 tokens -> [P, NIC*C] bf16 in (tile, ic)-block layout."""
    C = sum(tt for _, tt, _ in tile_spec)
    n = flat_rows.shape[0]
    xT = np.zeros((D, C), dtype=np.float32)
    if n:
        xT[:, :n] = flat_rows.T
    xP = np.empty((P, NIC * C), dtype=BF16NP)
    for t0, tt, _ in tile_spec:
        base = NIC * t0
        for ic in range(NIC):
            xP[:, base + ic * tt : base + (ic + 1) * tt] = xT[
                ic * P : (ic + 1) * P, t0 : t0 + tt
            ].astype(BF16NP)
    return xP


def _pack_w(wTs) -> np.ndarray:
    """list of [D, D] fp32 w.T -> [P, len*NIC*D] bf16, phase-major."""
    PHW = NIC * FB
    wP = np.empty((P, len(wTs) * NIC * D), dtype=BF16NP)
    for ws, wT in enumerate(wTs):
        base = ws * NIC * D
        for ph in range(NPH):
            for ic in range(NIC):
                wP[:, base + ph * PHW + ic * FB : base + ph * PHW + (ic + 1) * FB] = (
                    wT[ic * P : (ic + 1) * P, ph * FB : (ph + 1) * FB].astype(BF16NP)
                )
    return wP


def _pack_slots(block_counts):
    """DFS-pack per-expert block counts into 8 cores x slots (8,4,3).
    Returns per-expert list of (core, slot_idx, capacity_tokens), or None."""
    order = sorted(range(len(block_counts)), key=lambda i: -block_counts[i])

    def combos(n):
        out = []
        for a in range(9):
            for bq in range(9):
                for c in range(9):
                    cap = 8 * a + 4 * bq + 3 * c
                    if cap >= n and cap - n <= 6:
                        out.append((cap - n, a, bq, c))
        out.sort()
        return [(a, bq, c) for _, a, bq, c in out]

    assign = {}

    def dfs(k, rem):
        if k == len(order):
            return True
        n = block_counts[order[k]]
        if n == 0:
            assign[order[k]] = (0, 0, 0)
            return dfs(k + 1, rem)
        for a, bq, c in combos(n):
            if a <= rem[0] and bq <= rem[1] and c <= rem[2]:
                assign[order[k]] = (a, bq, c)
                if dfs(k + 1, (rem[0] - a, rem[1] - bq, rem[2] - c)):
                    return True
        return False

    if not dfs(0, (8, 8, 8)):
        return None
    free = {s: list(range(8)) for s in range(3)}
    pieces = [[] for _ in block_counts]
    for e in order:
        a, bq, c = assign[e]
        for s, cnt in ((0, a), (1, bq), (2, c)):
            for _ in range(cnt):
                core = free[s].pop(0)
                pieces[e].append((core, s, SLOT_BLOCKS[s] * P))
    return pieces


def _run(nc, in_maps, trace):
    return run_bass_kernel_spmd(nc, in_maps, list(range(N_CORES)), trace=trace)


def kernel(actions, action_type, W, b, _trace=False):
    actions = np.ascontiguousarray(actions, dtype=np.float32)
    B, L, _ = actions.shape
    flat = actions.reshape(B * L, D)
    types = np.asarray(action_type).reshape(B * L).astype(np.int64)

    idx = [np.flatnonzero(types == t) for t in range(N_CORES)]
    counts = [len(i) for i in idx]

    W = np.asarray(W, dtype=np.float32)
    b_np = np.asarray(b, dtype=np.float32)
    with_bias = bool(np.any(b_np))
    wTs = [np.eye(D, dtype=np.float32)] + [W[t].T for t in range(N_CORES - 1)]

    blocks = [-(-counts[t] // P) for t in range(1, N_CORES)]
    pieces = _pack_slots(blocks) if sum(blocks) <= 120 else None

    out_flat = np.empty_like(flat)
    out_flat[idx[0]] = flat[idx[0]]  # identity tokens: exact copy
    host_leftover = []  # (expert t, token indices) computed on host

    if pieces is not None:
        # Balanced path: all 8 cores, segments A/B/C per core.
        seg_off = {0: SEGS[0][0], 1: SEGS[1][0], 2: SEGS[2][0]}
        core_rows = [np.zeros((C_BAL, D), np.float32) for _ in range(N_CORES)]
        core_wseg = [[0, 0, 0] for _ in range(N_CORES)]  # wT index per segment
        core_orig = [np.full(C_BAL, -1, np.int64) for _ in range(N_CORES)]
        for t in range(1, N_CORES):
            toks = idx[t]
            pos = 0
            for core, s, cap in pieces[t - 1]:
                take = min(cap, len(toks) - pos)
                if take <= 0:
                    continue
                o = seg_off[s]
                core_rows[core][o : o + take] = flat[toks[pos : pos + take]]
                core_orig[core][o : o + take] = toks[pos : pos + take]
                core_wseg[core][s] = t
                pos += take
            if pos < len(toks):
                host_leftover.append((t, toks[pos:]))
        tile_spec = tuple(SEGS)
        nc = build_program(tile_spec, 3, with_bias)
        in_maps = []
        for core in range(N_CORES):
            m = {
                "xP": _pack_x(core_rows[core], tile_spec),
                "wP": _pack_w([wTs[core_wseg[core][s]] for s in range(3)]),
            }
            if with_bias:
                bb = np.zeros((P, 3 * D), np.float32)
                for s in range(3):
                    t = core_wseg[core][s]
                    if t >= 1:
                        bb[:, s * D : (s + 1) * D] = b_np[t - 1]
                m["bB"] = bb
            in_maps.append(m)
        r = _run(nc, in_maps, _trace)
        for core in range(N_CORES):
            o = (
                r.results[core]["outP"]
                .reshape(P, C_BAL // P, D)
                .transpose(1, 0, 2)
                .reshape(C_BAL, D)
            )
            valid = core_orig[core] >= 0
            out_flat[core_orig[core][valid]] = o[valid].astype(np.float32)
    else:
        # Fallback: one expert per core, core 0 runs dummy zeros.
        C = max(P, min(2048, -(-max(counts[1:]) // P) * P))
        tile_spec = tuple(_t_tiles(C))
        nc = build_program(tile_spec, 1, with_bias)
        in_maps = []
        for t in range(N_CORES):
            n_dev = 0 if t == 0 else min(counts[t], C)
            rows = np.zeros((C, D), np.float32)
            if n_dev:
                rows[:n_dev] = flat[idx[t][:n_dev]]
            m = {"xP": _pack_x(rows, tile_spec), "wP": _pack_w([wTs[t]])}
            if with_bias:
                bvec = np.zeros(D, dtype=np.float32) if t == 0 else b_np[t - 1]
                m["bB"] = np.ascontiguousarray(
                    np.broadcast_to(bvec, (P, D)), dtype=np.float32
                )
            in_maps.append(m)
            if t >= 1 and counts[t] > n_dev:
                host_leftover.append((t, idx[t][n_dev:]))
        r = _run(nc, in_maps, _trace)
        for t in range(1, N_CORES):
            n_dev = min(counts[t], C)
            if n_dev:
                o = (
                    r.results[t]["outP"]
                    .reshape(P, C // P, D)
                    .transpose(1, 0, 2)
                    .reshape(C, D)
                )
                out_flat[idx[t][:n_dev]] = o[:n_dev].astype(np.float32)

    for t, ov in host_leftover:
        out_flat[ov] = flat[ov] @ W[t - 1].T + b_np[t - 1]

    out = out_flat.reshape(B, L, D)
    if _trace:
        return out, r
    return out
